# revision 11
# baseline (speedup 1.0000x reference)
"""Additive (Bahdanau) attention kernel for 8 Trainium2 NeuronCores.

Reference computation (per (b,h) block, Lq=Lk=256, dk=64):
    qp = q @ Wq_w.T + Wq_b
    kp = k @ Wk_w.T + Wk_b
    scores[q,k] = vs_w . tanh(qp[q,:] + kp[k,:]) + vs_b
    attn = softmax(scores, axis=k)
    out  = attn @ v
Returns (out, attn).

Key observations:
  * vs_b is constant along k, so softmax cancels it exactly -> dropped.
  * |scores| <= sum|vs_w| <= 8, so exp() needs no max-subtraction.
  * B*H = 32 independent blocks -> 4 per core, params replicated, no
    collectives.

Device layout (per block): partitions = 128 = 2 x dk, packing a pair of
query positions (2j, 2j+1) per tile; k runs along the free dim.
  - kp2 [128, 256]  : kp^T duplicated in both partition halves
  - qp2 [128, 128]  : column j = [qp^T[:,2j] ; qp^T[:,2j+1]]
  - DVE tensor_scalar add (bf16, 4x mode) builds pre = kp2 + qp2[:,j]
  - ACT does one big tanh per J-pair batch (the throughput floor)
  - PE reduces over dk with stationary=tanh tile (bf16, FWL) and
    moving=vs2 [128,2] block-diagonal vs_w -> scoresT [k, q] in PSUM
  - softmax over k (partitions) via ones-matmul column sums
  - out = attnT.T @ v needs no transpose since attnT already has k on
    partitions; the host transposes attnT -> attn at the end.
"""

import os
from contextlib import ExitStack

import numpy as np

os.environ.setdefault("MYCRO_LOCAL_CACHE", "1")

import concourse.bass as bass
import concourse.bacc as bacc
import concourse.tile as tile
from concourse import mybir
from concourse.bass_utils import run_bass_kernel_spmd

F32 = mybir.dt.float32
BF16 = mybir.dt.bfloat16
AF = mybir.ActivationFunctionType

B, H, LQ, LK, DK = 4, 8, 256, 256, 64
NCORES = 8
NBLK = (B * H) // NCORES  # blocks per core
NPAIR = LQ // 2  # q-pairs per block
JB = 64  # q-pairs per tanh batch
NJB = NPAIR // JB  # batches per block


def build_nc(nblk=NBLK):
    nc = bacc.Bacc(None, target_bir_lowering=False)
    qT_d = nc.declare_dram_parameter("qT", [nblk, DK, LQ], F32, isOutput=False)
    kT_d = nc.declare_dram_parameter("kT", [nblk, DK, LK], F32, isOutput=False)
    v_d = nc.declare_dram_parameter("v", [nblk, LK, DK], F32, isOutput=False)
    WqT_d = nc.declare_dram_parameter("WqT", [DK, DK], F32, isOutput=False)
    Wqb_d = nc.declare_dram_parameter("Wqb", [DK, 1], F32, isOutput=False)
    WkT_d = nc.declare_dram_parameter("WkT", [DK, DK], F32, isOutput=False)
    Wkb_d = nc.declare_dram_parameter("Wkb", [DK, 1], F32, isOutput=False)
    vsc_d = nc.declare_dram_parameter("vsc", [DK, 1], F32, isOutput=False)
    out_d = nc.declare_dram_parameter("out", [nblk, LQ, DK], F32, isOutput=True)
    attnT_d = nc.declare_dram_parameter("attnT", [nblk, LK, LQ], F32, isOutput=True)

    with ExitStack() as ctx:
        tc = ctx.enter_context(tile.TileContext(nc))
        consts = ctx.enter_context(tc.tile_pool(name="consts", bufs=1))
        proj_in = ctx.enter_context(tc.tile_pool(name="proj_in", bufs=2))
        blk = ctx.enter_context(tc.tile_pool(name="blk", bufs=2))
        pre_pool = ctx.enter_context(tc.tile_pool(name="pre", bufs=2))
        tanh_pool = ctx.enter_context(tc.tile_pool(name="tanh", bufs=2))
        soft = ctx.enter_context(tc.tile_pool(name="soft", bufs=2))
        ps_proj = ctx.enter_context(tc.tile_pool(name="ps_proj", bufs=1, space="PSUM"))
        ps_scores = ctx.enter_context(
            tc.tile_pool(name="ps_scores", bufs=2, space="PSUM")
        )
        ps_small = ctx.enter_context(tc.tile_pool(name="ps_small", bufs=1, space="PSUM"))
        ps_out = ctx.enter_context(tc.tile_pool(name="ps_out", bufs=2, space="PSUM"))

        # ---- constants ----
        WqT_sb = consts.tile([DK, DK], F32)
        nc.sync.dma_start(out=WqT_sb, in_=WqT_d[:, :])
        WkT_sb = consts.tile([DK, DK], F32)
        nc.sync.dma_start(out=WkT_sb, in_=WkT_d[:, :])
        Wqb_sb = consts.tile([DK, 1], F32)
        nc.sync.dma_start(out=Wqb_sb, in_=Wqb_d[:, :])
        Wkb_sb = consts.tile([DK, 1], F32)
        nc.sync.dma_start(out=Wkb_sb, in_=Wkb_d[:, :])
        vsc_sb = consts.tile([DK, 1], F32)
        nc.sync.dma_start(out=vsc_sb, in_=vsc_d[:, :])

        # vs2 [128, 2] block diagonal: col0 = [vs;0], col1 = [0;vs]
        vs2 = consts.tile([128, 2], BF16)
        nc.vector.memset(vs2, 0.0)
        nc.vector.tensor_copy(vs2[0:DK, 0:1], vsc_sb)
        nc.vector.tensor_copy(vs2[DK : 2 * DK, 1:2], vsc_sb)

        ones_m = consts.tile([1, 128], BF16)  # lhsT for recip broadcast (K=1)
        nc.vector.memset(ones_m, 1.0)
        ones_k = consts.tile([128, 1], BF16)  # lhsT for column sums (M=1)
        nc.vector.memset(ones_k, 1.0)

        for b in range(nblk):
            # ---- load + project q,k ----
            qT_sb = proj_in.tile([DK, LQ], F32, tag="qT_sb")
            nc.sync.dma_start(out=qT_sb, in_=qT_d[b])
            kT_sb = proj_in.tile([DK, LK], F32, tag="kT_sb")
            nc.sync.dma_start(out=kT_sb, in_=kT_d[b])
            v_sb = proj_in.tile([128, 2, DK], F32, tag="v_sb")
            nc.sync.dma_start(
                out=v_sb, in_=v_d[b].rearrange("(h p) d -> p h d", p=128)
            )

            # qp^T [e, q] = WqT.T @ qT ; then +bias, pack pairs into qp2
            ps_qp = ps_proj.tile([DK, LQ], F32, tag="ps_proj")
            nc.tensor.matmul(ps_qp, lhsT=WqT_sb, rhs=qT_sb, start=True, stop=True)
            qp2 = blk.tile([128, NPAIR], F32, tag="qp2")
            # even q -> partitions 0:64, odd q -> partitions 64:128
            nc.vector.tensor_scalar_add(qp2[0:DK, :], ps_qp[:, 0:LQ:2], Wqb_sb)
            nc.vector.tensor_scalar_add(qp2[DK : 2 * DK, :], ps_qp[:, 1:LQ:2], Wqb_sb)

            ps_kp = ps_proj.tile([DK, LK], F32, tag="ps_proj")
            nc.tensor.matmul(ps_kp, lhsT=WkT_sb, rhs=kT_sb, start=True, stop=True)
            kp2 = blk.tile([128, LK], BF16, tag="kp2")
            nc.vector.tensor_scalar_add(kp2[0:DK, :], ps_kp, Wkb_sb)
            nc.vector.tensor_scalar_add(kp2[DK : 2 * DK, :], ps_kp, Wkb_sb)

            # ---- main loop: scoresT[k, q] ----
            # one full PSUM bank: cols [h*256 + q] for k-half h
            ps_sc = ps_scores.tile([128, 2 * LQ], F32, tag="ps_scores")
            for jb in range(NJB):
                pre = pre_pool.tile([128, JB * LK], BF16, tag="pre")
                for jj in range(JB):
                    j = jb * JB + jj
                    nc.vector.tensor_scalar_add(
                        pre[:, jj * LK : (jj + 1) * LK], kp2, qp2[:, j : j + 1]
                    )
                th = tanh_pool.tile([128, JB * LK], BF16, tag="tanh")
                nc.scalar.activation(th, pre, AF.Tanh)
                for jj in range(JB):
                    j = jb * JB + jj
                    for h in range(2):
                        nc.tensor.matmul(
                            ps_sc[:, h * LQ + 2 * j : h * LQ + 2 * j + 2],
                            lhsT=th[:, jj * LK + h * 128 : jj * LK + (h + 1) * 128],
                            rhs=vs2,
                            start=True,
                            stop=True,
                        )

            # ---- softmax over k (partitions) ----
            exp_sb = soft.tile([128, 2 * LQ], BF16, tag="exp_sb")
            nc.scalar.activation(exp_sb, ps_sc, AF.Exp)
            ps_sum = ps_small.tile([1, LQ], F32, tag="ps_sum")
            nc.tensor.matmul(
                ps_sum, lhsT=ones_k, rhs=exp_sb[:, 0:LQ], start=True, stop=False
            )
            nc.tensor.matmul(
                ps_sum, lhsT=ones_k, rhs=exp_sb[:, LQ : 2 * LQ], start=False, stop=True
            )
            recip_f = soft.tile([1, LQ], F32, tag="recip_f")
            nc.vector.reciprocal(recip_f, ps_sum)
            recip_bf = soft.tile([1, LQ], BF16, tag="recip_bf")
            nc.vector.tensor_copy(recip_bf, recip_f)
            ps_bc = ps_small.tile([128, LQ], F32, tag="ps_bc")
            nc.tensor.matmul(ps_bc, lhsT=ones_m, rhs=recip_bf, start=True, stop=True)
            recipB = soft.tile([128, LQ], BF16, tag="recipB")
            nc.vector.tensor_copy(recipB, ps_bc)

            # attnT[k, q] = exp * (1/sum), fp32 for output + attn@v weights
            attnT_f = soft.tile([128, 2, LQ], F32, tag="attnT_f")
            for h in range(2):
                nc.vector.tensor_mul(
                    attnT_f[:, h, :], exp_sb[:, h * LQ : (h + 1) * LQ], recipB
                )
            nc.sync.dma_start(
                out=attnT_d[b].rearrange("(h p) q -> p h q", p=128), in_=attnT_f
            )

            # ---- out[q, d] = sum_k attnT[k, q] * v[k, d] ----
            for qc in range(2):
                ps_o = ps_out.tile([128, DK], F32, tag="ps_out")
                for h in range(2):
                    nc.tensor.matmul(
                        ps_o,
                        lhsT=attnT_f[:, h, qc * 128 : (qc + 1) * 128],
                        rhs=v_sb[:, h, :],
                        start=(h == 0),
                        stop=(h == 1),
                    )
                o_sb = soft.tile([128, DK], F32, tag="o_sb")
                nc.vector.tensor_copy(o_sb, ps_o)
                nc.sync.dma_start(
                    out=out_d[b, qc * 128 : (qc + 1) * 128, :], in_=o_sb
                )

    return nc


_CACHED = {}


def _get_nc(nblk=NBLK):
    if nblk not in _CACHED:
        nc = build_nc(nblk)
        nc.finalize()
        _CACHED[nblk] = nc
    return _CACHED[nblk]


def _prep_in_maps(q, k, v, Wq_w, Wq_b, Wk_w, Wk_b, vs_w):
    q3 = np.asarray(q, np.float32).reshape(B * H, LQ, DK)
    k3 = np.asarray(k, np.float32).reshape(B * H, LK, DK)
    v3 = np.asarray(v, np.float32).reshape(B * H, LK, DK)
    qT = np.ascontiguousarray(q3.transpose(0, 2, 1))
    kT = np.ascontiguousarray(k3.transpose(0, 2, 1))
    common = {
        "WqT": np.ascontiguousarray(np.asarray(Wq_w, np.float32).T),
        "Wqb": np.asarray(Wq_b, np.float32).reshape(DK, 1).copy(),
        "WkT": np.ascontiguousarray(np.asarray(Wk_w, np.float32).T),
        "Wkb": np.asarray(Wk_b, np.float32).reshape(DK, 1).copy(),
        "vsc": np.asarray(vs_w, np.float32).reshape(DK, 1).copy(),
    }
    in_maps = []
    for i in range(NCORES):
        s = slice(i * NBLK, (i + 1) * NBLK)
        in_maps.append(
            {
                "qT": np.ascontiguousarray(qT[s]),
                "kT": np.ascontiguousarray(kT[s]),
                "v": np.ascontiguousarray(v3[s]),
                **common,
            }
        )
    return in_maps


def _run(inputs, trace=False):
    nc = _get_nc()
    in_maps = _prep_in_maps(
        inputs["q"], inputs["k"], inputs["v"], inputs["Wq_w"], inputs["Wq_b"],
        inputs["Wk_w"], inputs["Wk_b"], inputs["vs_w"],
    )
    res = run_bass_kernel_spmd(nc, in_maps, list(range(NCORES)), trace=trace)
    outs = np.stack([res.results[i]["out"] for i in range(NCORES)])
    attnTs = np.stack([res.results[i]["attnT"] for i in range(NCORES)])
    output = outs.reshape(B, H, LQ, DK)
    attn = attnTs.reshape(B * H, LK, LQ).transpose(0, 2, 1).reshape(B, H, LQ, LK)
    return (output, np.ascontiguousarray(attn)), res


def kernel(q, k, v, Wq_w, Wq_b, Wk_w, Wk_b, vs_w, vs_b):
    (output, attn), _ = _run(
        {
            "q": q, "k": k, "v": v, "Wq_w": Wq_w, "Wq_b": Wq_b,
            "Wk_w": Wk_w, "Wk_b": Wk_b, "vs_w": vs_w,
        }
    )
    return output, attn


# revision 19
# speedup vs baseline: 1.1408x; 1.1408x over previous
"""Additive (Bahdanau) attention kernel for 8 Trainium2 NeuronCores.

Reference computation (per (b,h) block, Lq=Lk=256, dk=64):
    qp = q @ Wq_w.T + Wq_b
    kp = k @ Wk_w.T + Wk_b
    scores[q,k] = vs_w . tanh(qp[q,:] + kp[k,:]) + vs_b
    attn = softmax(scores, axis=k)
    out  = attn @ v
Returns (out, attn).

Key observations:
  * vs_b is constant along k, so softmax cancels it exactly -> dropped.
  * |scores| <= sum|vs_w| <= 8, so exp() needs no max-subtraction.
  * B*H = 32 independent blocks -> 4 per core, params replicated, no
    collectives.

Device layout (per block): partitions = 128 = 2 x dk, packing the query
pair (j, j+128) per column; k runs along the free dim.
  - kp2 [128, 256]  : kp^T duplicated in both partition halves
  - qp2 [128, 128]  : column j = [qp^T[:,j] ; qp^T[:,j+128]]
  - DVE tensor_scalar add builds pre[:, j-slice] = kp2 + qp2[:,j]
  - ACT does one big tanh per J-pair batch (a few pairs instead run
    fused add+tanh on ACT via the per-partition bias, to balance engines)
  - PE reduces over dk with stationary=tanh tile (bf16, FWL) and
    moving=vs2 [128,2] block-diagonal vs_w -> scoresT [k, q] in PSUM
  - softmax over k (partitions) via ones-matmul column sums
  - out = attnT.T @ v needs no transpose since attnT already has k on
    partitions; the host transposes attnT -> attn at the end.
"""

import os
from contextlib import ExitStack

import numpy as np

os.environ.setdefault("MYCRO_LOCAL_CACHE", "1")

import concourse.bass as bass
import concourse.bacc as bacc
import concourse.tile as tile
from concourse import mybir
from concourse.bass_utils import run_bass_kernel_spmd

F32 = mybir.dt.float32
BF16 = mybir.dt.bfloat16
AF = mybir.ActivationFunctionType

B, H, LQ, LK, DK = 4, 8, 256, 256, 64
NCORES = 8
NBLK = (B * H) // NCORES  # blocks per core
NPAIR = LQ // 2  # q-pairs per block; pair j = queries (j, j+128)
NFUSE = 0  # pairs per block computed as fused add+tanh on ACT
JB = 16  # q-pairs per DVE/tanh batch
NJB = (NPAIR - NFUSE) // JB  # DVE-path batches per block
assert NJB * JB + NFUSE == NPAIR


def build_nc(nblk=NBLK):
    nc = bacc.Bacc(None, target_bir_lowering=False)
    # qkT[b]: [128, 256] = [qT (d x Lq) ; kT (d x Lk)] stacked on partitions
    qkT_d = nc.declare_dram_parameter("qkT", [nblk, 2 * DK, LQ], F32, isOutput=False)
    v_d = nc.declare_dram_parameter("v", [nblk, LK, DK], F32, isOutput=False)
    # par[0:64, 0:64]=WqT, par[64:128, 0:64]=WkT (so each projection matmul
    # has lhsT/rhs at the same base partition); par[0:64, 64]=Wqb,
    # par[0:64, 65]=Wkb, par[0:64, 66]=vs
    par_d = nc.declare_dram_parameter("par", [2 * DK, DK + 3], F32, isOutput=False)
    out_d = nc.declare_dram_parameter("out", [nblk, LQ, DK], F32, isOutput=True)
    attnT_d = nc.declare_dram_parameter("attnT", [nblk, LK, LQ], F32, isOutput=True)

    with ExitStack() as ctx:
        tc = ctx.enter_context(tile.TileContext(nc))
        consts = ctx.enter_context(tc.tile_pool(name="consts", bufs=1))
        proj_in = ctx.enter_context(tc.tile_pool(name="proj_in", bufs=2))
        blk = ctx.enter_context(tc.tile_pool(name="blk", bufs=2))
        pre_pool = ctx.enter_context(tc.tile_pool(name="pre", bufs=3))
        tanh_pool = ctx.enter_context(tc.tile_pool(name="tanh", bufs=3))
        soft = ctx.enter_context(tc.tile_pool(name="soft", bufs=2))
        ps_proj = ctx.enter_context(tc.tile_pool(name="ps_proj", bufs=1, space="PSUM"))
        ps_scores = ctx.enter_context(
            tc.tile_pool(name="ps_scores", bufs=2, space="PSUM")
        )
        ps_small = ctx.enter_context(tc.tile_pool(name="ps_small", bufs=1, space="PSUM"))
        ps_out = ctx.enter_context(tc.tile_pool(name="ps_out", bufs=2, space="PSUM"))

        # ---- constants (single DMA) ----
        par_sb = consts.tile([2 * DK, DK + 3], F32)
        nc.sync.dma_start(out=par_sb, in_=par_d[:, :])
        WqT_sb = par_sb[0:DK, 0:DK]
        WkT_sb = par_sb[DK : 2 * DK, 0:DK]
        Wqb_sb = par_sb[0:DK, DK : DK + 1]
        Wkb_sb = par_sb[0:DK, DK + 1 : DK + 2]
        vsc_sb = par_sb[0:DK, DK + 2 : DK + 3]

        # vs2 [128, 2] block diagonal: col0 = [vs;0], col1 = [0;vs]
        vs2 = consts.tile([128, 2], BF16)
        nc.vector.memset(vs2, 0.0)
        nc.vector.tensor_copy(vs2[0:DK, 0:1], vsc_sb)
        nc.vector.tensor_copy(vs2[DK : 2 * DK, 1:2], vsc_sb)

        ones_m = consts.tile([1, 128], BF16)  # lhsT for recip broadcast (K=1)
        nc.vector.memset(ones_m, 1.0)
        ones_k = consts.tile([128, 1], BF16)  # lhsT for column sums (M=1)
        nc.vector.memset(ones_k, 1.0)

        for b in range(nblk):
            # ---- load + project q,k ----
            qkT_sb = proj_in.tile([2 * DK, LQ], F32, tag="qkT_sb")
            nc.sync.dma_start(out=qkT_sb, in_=qkT_d[b])
            v_sb = proj_in.tile([128, 2, DK], F32, tag="v_sb")
            nc.sync.dma_start(
                out=v_sb, in_=v_d[b].rearrange("(h p) d -> p h d", p=128)
            )

            # qp^T [e, q] = WqT.T @ qT ; +bias, pack pairs (j, j+128) into qp2
            ps_qp = ps_proj.tile([DK, LQ], F32, tag="ps_proj")
            nc.tensor.matmul(
                ps_qp, lhsT=WqT_sb, rhs=qkT_sb[0:DK, :], start=True, stop=True
            )
            qp2 = blk.tile([128, NPAIR], F32, tag="qp2")
            nc.vector.tensor_scalar_add(qp2[0:DK, :], ps_qp[:, 0:LQ:2], Wqb_sb)
            nc.vector.tensor_scalar_add(qp2[DK : 2 * DK, :], ps_qp[:, 1:LQ:2], Wqb_sb)

            ps_kp = ps_proj.tile([DK, LK], F32, tag="ps_proj")
            nc.tensor.matmul(
                ps_kp, lhsT=WkT_sb, rhs=qkT_sb[DK : 2 * DK, :], start=True, stop=True
            )
            kp2 = blk.tile([128, LK], BF16, tag="kp2")
            nc.vector.tensor_scalar_add(kp2[0:DK, :], ps_kp, Wkb_sb)
            nc.vector.tensor_scalar_add(kp2[DK : 2 * DK, :], ps_kp, Wkb_sb)

            # ---- main loop: scoresT[k, q] ----
            # one full PSUM bank: col h*256 + q for k-half h; pair j ->
            # columns (j, j+128) within each half.
            ps_sc = ps_scores.tile([128, 2 * LQ], F32, tag="ps_scores")

            def scores_mm(th_ap, j):
                # pair j = queries (2j, 2j+1) -> adjacent cols in each k-half
                for h in range(2):
                    nc.tensor.matmul(
                        ps_sc[:, h * LQ + 2 * j : h * LQ + 2 * j + 2],
                        lhsT=th_ap[:, h * 128 : (h + 1) * 128],
                        rhs=vs2,
                        start=True,
                        stop=True,
                    )

            for jb in range(NJB):
                pre = pre_pool.tile([128, JB * LK], BF16, tag="pre")
                for jj in range(JB):
                    j = jb * JB + jj
                    nc.vector.tensor_scalar_add(
                        pre[:, jj * LK : (jj + 1) * LK], kp2, qp2[:, j : j + 1]
                    )
                th = tanh_pool.tile([128, JB * LK], BF16, tag="tanh")
                nc.scalar.activation(th, pre, AF.Tanh)
                for jj in range(JB):
                    j = jb * JB + jj
                    scores_mm(th[:, jj * LK : (jj + 1) * LK], j)

            # fused pairs: ACT does tanh(kp2 + qp2[:, j]) directly
            if NFUSE:
                thf = tanh_pool.tile([128, NFUSE * LK], BF16, tag="tanh_f")
                for i in range(NFUSE):
                    j = NJB * JB + i
                    nc.scalar.activation(
                        thf[:, i * LK : (i + 1) * LK],
                        kp2,
                        AF.Tanh,
                        bias=qp2[:, j : j + 1],
                    )
                for i in range(NFUSE):
                    j = NJB * JB + i
                    scores_mm(thf[:, i * LK : (i + 1) * LK], j)

            # ---- softmax over k (partitions) ----
            exp_sb = soft.tile([128, 2 * LQ], BF16, tag="exp_sb")
            nc.scalar.activation(exp_sb, ps_sc, AF.Exp)
            ps_sum = ps_small.tile([1, LQ], F32, tag="ps_sum")
            nc.tensor.matmul(
                ps_sum, lhsT=ones_k, rhs=exp_sb[:, 0:LQ], start=True, stop=False
            )
            nc.tensor.matmul(
                ps_sum, lhsT=ones_k, rhs=exp_sb[:, LQ : 2 * LQ], start=False, stop=True
            )
            recip_f = soft.tile([1, LQ], F32, tag="recip_f")
            nc.vector.reciprocal(recip_f, ps_sum)
            recip_bf = soft.tile([1, LQ], BF16, tag="recip_bf")
            nc.vector.tensor_copy(recip_bf, recip_f)
            ps_bc = ps_small.tile([128, LQ], F32, tag="ps_bc")
            nc.tensor.matmul(ps_bc, lhsT=ones_m, rhs=recip_bf, start=True, stop=True)
            recipB = soft.tile([128, LQ], BF16, tag="recipB")
            nc.vector.tensor_copy(recipB, ps_bc)

            # attnT[k, q] = exp * (1/sum), fp32 for output + attn@v weights
            attnT_f = soft.tile([128, 2, LQ], F32, tag="attnT_f")
            for h in range(2):
                nc.vector.tensor_mul(
                    attnT_f[:, h, :], exp_sb[:, h * LQ : (h + 1) * LQ], recipB
                )
            nc.sync.dma_start(
                out=attnT_d[b].rearrange("(h p) q -> p h q", p=128), in_=attnT_f
            )

            # ---- out[q, d] = sum_k attnT[k, q] * v[k, d] ----
            for qc in range(2):
                ps_o = ps_out.tile([128, DK], F32, tag="ps_out")
                for h in range(2):
                    nc.tensor.matmul(
                        ps_o,
                        lhsT=attnT_f[:, h, qc * 128 : (qc + 1) * 128],
                        rhs=v_sb[:, h, :],
                        start=(h == 0),
                        stop=(h == 1),
                    )
                o_sb = soft.tile([128, DK], F32, tag="o_sb")
                nc.vector.tensor_copy(o_sb, ps_o)
                nc.sync.dma_start(
                    out=out_d[b, qc * 128 : (qc + 1) * 128, :], in_=o_sb
                )

    return nc


_CACHED = {}


def _get_nc(nblk=NBLK):
    if nblk not in _CACHED:
        nc = build_nc(nblk)
        nc.finalize()
        _CACHED[nblk] = nc
    return _CACHED[nblk]


def _prep_in_maps(q, k, v, Wq_w, Wq_b, Wk_w, Wk_b, vs_w):
    q3 = np.asarray(q, np.float32).reshape(B * H, LQ, DK)
    k3 = np.asarray(k, np.float32).reshape(B * H, LK, DK)
    v3 = np.asarray(v, np.float32).reshape(B * H, LK, DK)
    qkT = np.concatenate(
        [q3.transpose(0, 2, 1), k3.transpose(0, 2, 1)], axis=1
    )  # [B*H, 128, 256]
    par = np.zeros((2 * DK, DK + 3), np.float32)
    par[0:DK, 0:DK] = np.asarray(Wq_w, np.float32).T
    par[DK : 2 * DK, 0:DK] = np.asarray(Wk_w, np.float32).T
    par[0:DK, DK] = np.asarray(Wq_b, np.float32).reshape(DK)
    par[0:DK, DK + 1] = np.asarray(Wk_b, np.float32).reshape(DK)
    par[0:DK, DK + 2] = np.asarray(vs_w, np.float32).reshape(DK)
    in_maps = []
    for i in range(NCORES):
        s = slice(i * NBLK, (i + 1) * NBLK)
        in_maps.append(
            {
                "qkT": np.ascontiguousarray(qkT[s]),
                "v": np.ascontiguousarray(v3[s]),
                "par": par,
            }
        )
    return in_maps


def _run(inputs, trace=False):
    nc = _get_nc()
    in_maps = _prep_in_maps(
        inputs["q"], inputs["k"], inputs["v"], inputs["Wq_w"], inputs["Wq_b"],
        inputs["Wk_w"], inputs["Wk_b"], inputs["vs_w"],
    )
    res = run_bass_kernel_spmd(nc, in_maps, list(range(NCORES)), trace=trace)
    outs = np.stack([res.results[i]["out"] for i in range(NCORES)])
    attnTs = np.stack([res.results[i]["attnT"] for i in range(NCORES)])
    output = outs.reshape(B, H, LQ, DK)
    attn = attnTs.reshape(B * H, LK, LQ).transpose(0, 2, 1).reshape(B, H, LQ, LK)
    return (output, np.ascontiguousarray(attn)), res


def kernel(q, k, v, Wq_w, Wq_b, Wk_w, Wk_b, vs_w, vs_b):
    (output, attn), _ = _run(
        {
            "q": q, "k": k, "v": v, "Wq_w": Wq_w, "Wq_b": Wq_b,
            "Wk_w": Wk_w, "Wk_b": Wk_b, "vs_w": vs_w,
        }
    )
    return output, attn
